# revision 6
# baseline (speedup 1.0000x reference)
"""DistMult edge scoring on 8 Trainium2 NeuronCores.

score[r, e] = sum_d x[src[r,e], d] * x[dst[r,e], d] * rel[r, d]

Strategy (edge-sharded, data-parallel):
  - Shard the 500k edges across 8 cores (62500 each); replicate the node
    table (51 MB) and rel embeddings in each core's DRAM.
  - Gathers use the batched GPSIMD dma_gather (one 512 B descriptor per
    row). Its indices are int16, so the host buckets each core's edges by
    (src_range, dst_range) pairs of 32768-row table ranges; within a
    bucket both gathers draw from a single range with local int16 indices.
  - Buckets are padded to the max size across cores (rounded to 128) so a
    single SPMD program serves all 8 cores; pad slots gather row 0 of the
    range and their scores are discarded on the host.
  - Per chunk of up to 4096 edges: gather xs, xd; DVE: v = xs*xd,
    u = v*rel (0-stride broadcast), grouped reduce_sum -> scores.
  - Chunk element t lands at (partition t%128, column t//128); the host
    inverts the whole permutation when assembling the output.
"""

import numpy as np

N_CORES = 8
N_NODES = 100000
DIM = 128
N_REL = 3
N_EDGES = 500000
E_CORE = N_EDGES // N_CORES          # 62500
RANGE = 32768                        # int16-addressable table range
N_RANGES = (N_NODES + RANGE - 1) // RANGE   # 4
N_BUCKETS = N_RANGES * N_RANGES      # 16
CHUNK_COLS = 32                      # max columns (128 edges each) per gather

_CACHE = {}
LAST_RESULTS = None  # bass_utils.BassKernelResults from the most recent run


def _build_nc(caps, cols_max):
    """caps[r][k] = padded bucket capacity (multiple of 128) for relation r,
    bucket k = src_range*N_RANGES + dst_range. cols_max = max_r total cols."""
    from contextlib import ExitStack

    import concourse.bacc as bacc
    import concourse.mybir as mybir
    import concourse.tile as tile

    tot_s = sum(sum(c) for c in caps) // 16  # int16 image cols per side

    nc = bacc.Bacc("TRN2", target_bir_lowering=False, debug=False,
                   num_devices=N_CORES)
    table = nc.dram_tensor("table", (N_NODES, DIM), mybir.dt.float32,
                           kind="ExternalInput")
    relb = nc.dram_tensor("relb", (N_REL, 128, DIM), mybir.dt.float32,
                          kind="ExternalInput")
    sidx = nc.dram_tensor("sidx", (128, tot_s), mybir.dt.int16,
                          kind="ExternalInput")
    didx = nc.dram_tensor("didx", (128, tot_s), mybir.dt.int16,
                          kind="ExternalInput")
    out = nc.dram_tensor("out", (N_REL, 128, cols_max), mybir.dt.float32,
                         kind="ExternalOutput")

    with tile.TileContext(nc) as tc, ExitStack() as ctx:
        const_pool = ctx.enter_context(tc.tile_pool(name="const", bufs=2))
        ipool = ctx.enter_context(tc.tile_pool(name="idx", bufs=4))
        gpool = ctx.enter_context(tc.tile_pool(name="gather", bufs=3))
        spool = ctx.enter_context(tc.tile_pool(name="scores", bufs=2))

        img_off = 0  # running column offset into the int16 index images
        for r in range(N_REL):
            relb_sb = const_pool.tile([128, DIM], mybir.dt.float32, tag="relb")
            nc.sync.dma_start(out=relb_sb[:], in_=relb[r, :, :])
            cols_r = sum(caps[r]) // 128
            scores_sb = spool.tile([128, cols_max], mybir.dt.float32,
                                   tag="scores")
            col_off = 0
            for k in range(N_BUCKETS):
                cap = caps[r][k]
                if cap == 0:
                    continue
                si = (k // N_RANGES) * RANGE
                di = (k % N_RANGES) * RANGE
                s_tab = table[si : min(si + RANGE, N_NODES), :]
                d_tab = table[di : min(di + RANGE, N_NODES), :]
                cols_b = cap // 128
                for c0 in range(0, cols_b, CHUNK_COLS):
                    cc = min(CHUNK_COLS, cols_b - c0)
                    n = cc * 128
                    s_im = ipool.tile([128, CHUNK_COLS * 8], mybir.dt.int16,
                                      tag="sidx")
                    d_im = ipool.tile([128, CHUNK_COLS * 8], mybir.dt.int16,
                                      tag="didx")
                    nc.sync.dma_start(out=s_im[:, : cc * 8],
                                      in_=sidx[:, img_off : img_off + cc * 8])
                    nc.sync.dma_start(out=d_im[:, : cc * 8],
                                      in_=didx[:, img_off : img_off + cc * 8])
                    img_off += cc * 8
                    xs_t = gpool.tile([128, CHUNK_COLS * DIM],
                                      mybir.dt.float32, tag="xs")
                    xd_t = gpool.tile([128, CHUNK_COLS * DIM],
                                      mybir.dt.float32, tag="xd")
                    nc.gpsimd.dma_gather(
                        xs_t[:, : cc * DIM].rearrange("p (c d) -> p c d",
                                                      d=DIM),
                        s_tab, s_im[:, : cc * 8], n, n, DIM,
                        single_packet=False)
                    nc.gpsimd.dma_gather(
                        xd_t[:, : cc * DIM].rearrange("p (c d) -> p c d",
                                                      d=DIM),
                        d_tab, d_im[:, : cc * 8], n, n, DIM,
                        single_packet=False)
                    nc.vector.tensor_tensor(
                        out=xd_t[:, : cc * DIM], in0=xs_t[:, : cc * DIM],
                        in1=xd_t[:, : cc * DIM], op=mybir.AluOpType.mult)
                    nc.vector.tensor_tensor(
                        out=xd_t[:, : cc * DIM], in0=xd_t[:, : cc * DIM],
                        in1=relb_sb[:, None, :].to_broadcast([128, cc, DIM]),
                        op=mybir.AluOpType.mult)
                    nc.vector.reduce_sum(
                        out=scores_sb[:, col_off : col_off + cc],
                        in_=xd_t[:, : cc * DIM].rearrange("p (c d) -> p c d",
                                                          d=DIM),
                        axis=mybir.AxisListType.X)
                    col_off += cc
            assert col_off == cols_r
            nc.sync.dma_start(out=out[r, :, :cols_r],
                              in_=scores_sb[:, :cols_r])

    nc.compile()
    return nc


def _pack_idx16(vals, cap):
    """Local indices [n] (n <= cap, cap % 128 == 0) -> replicated int16
    image [128, cap // 16], zero-padded."""
    a = np.zeros(cap, dtype=np.int16)
    a[: len(vals)] = vals
    return np.tile(a.reshape(-1, 16).T, (8, 1))  # [16, cap/16] -> [128, .]


def prepare(node_embeds, rel_emb, src_idx, dst_idx):
    """Bucket/pack host-side; returns (nc, in_maps, assemble) where
    assemble(results) -> full [N_REL, N_EDGES] scores."""
    node_embeds = np.ascontiguousarray(np.asarray(node_embeds,
                                                  dtype=np.float32))
    rel_emb = np.asarray(rel_emb, dtype=np.float32)
    src_idx = np.asarray(src_idx).astype(np.int64)
    dst_idx = np.asarray(dst_idx).astype(np.int64)

    relb = np.ascontiguousarray(
        np.broadcast_to(rel_emb[:, None, :], (N_REL, 128, DIM)))

    # ---- host-side bucketing ----
    # orders[c][r]: edge permutation (bucket-major); counts[c][r][k]
    orders = [[None] * N_REL for _ in range(N_CORES)]
    counts = np.zeros((N_CORES, N_REL, N_BUCKETS), dtype=np.int64)
    s_loc = [[None] * N_REL for _ in range(N_CORES)]
    d_loc = [[None] * N_REL for _ in range(N_CORES)]
    for c in range(N_CORES):
        lo = c * E_CORE
        for r in range(N_REL):
            s = src_idx[r, lo : lo + E_CORE]
            d = dst_idx[r, lo : lo + E_CORE]
            b = (s // RANGE) * N_RANGES + (d // RANGE)
            order = np.argsort(b, kind="stable")
            orders[c][r] = order
            counts[c, r] = np.bincount(b, minlength=N_BUCKETS)
            s_loc[c][r] = (s[order] % RANGE).astype(np.int16)
            d_loc[c][r] = (d[order] % RANGE).astype(np.int16)

    caps = [[int(-(-counts[:, r, k].max() // 128) * 128)
             for k in range(N_BUCKETS)] for r in range(N_REL)]
    cols_max = max(sum(caps[r]) for r in range(N_REL)) // 128

    key = (tuple(map(tuple, caps)), cols_max)
    if _CACHE.get("key") != key:
        _CACHE["nc"] = _build_nc(caps, cols_max)
        _CACHE["key"] = key
    nc = _CACHE["nc"]

    # ---- pack index images (chunked exactly like the device loop) ----
    tot_s = sum(sum(c) for c in caps) // 16
    in_maps = []
    for c in range(N_CORES):
        s_img = np.empty((128, tot_s), dtype=np.int16)
        d_img = np.empty((128, tot_s), dtype=np.int16)
        img_off = 0
        for r in range(N_REL):
            u = 0  # position within this core's bucket-sorted edge stream
            for k in range(N_BUCKETS):
                cap = caps[r][k]
                if cap == 0:
                    continue
                cnt = int(counts[c, r, k])
                sv = s_loc[c][r][u : u + cnt]
                dv = d_loc[c][r][u : u + cnt]
                u += cnt
                # pad bucket to cap, then emit in CHUNK_COLS chunks
                sp = np.zeros(cap, dtype=np.int16); sp[:cnt] = sv
                dp = np.zeros(cap, dtype=np.int16); dp[:cnt] = dv
                for c0 in range(0, cap // 128, CHUNK_COLS):
                    cc = min(CHUNK_COLS, cap // 128 - c0)
                    n = cc * 128
                    seg = slice(c0 * 128, c0 * 128 + n)
                    s_img[:, img_off : img_off + cc * 8] = _pack_idx16(sp[seg], n)
                    d_img[:, img_off : img_off + cc * 8] = _pack_idx16(dp[seg], n)
                    img_off += cc * 8
        assert img_off == tot_s
        in_maps.append({"table": node_embeds, "relb": relb,
                        "sidx": s_img, "didx": d_img})

    def assemble(results):
        out = np.empty((N_REL, N_EDGES), dtype=np.float32)
        for c, res in enumerate(results):
            buf = res["out"]  # [N_REL, 128, cols_max]
            lo = c * E_CORE
            for r in range(N_REL):
                colmajor = buf[r].T.ravel()  # index = col*128 + partition
                u = 0
                off = 0
                for k in range(N_BUCKETS):
                    cap = caps[r][k]
                    if cap == 0:
                        continue
                    cnt = int(counts[c, r, k])
                    e_ids = orders[c][r][u : u + cnt]
                    out[r, lo + e_ids] = colmajor[off : off + cnt]
                    u += cnt
                    off += cap
        return out

    return nc, in_maps, assemble


def kernel(node_embeds, rel_emb, src_idx, dst_idx):
    global LAST_RESULTS
    from concourse import bass_utils

    nc, in_maps, assemble = prepare(node_embeds, rel_emb, src_idx, dst_idx)
    LAST_RESULTS = bass_utils.run_bass_kernel_spmd(
        nc, in_maps, core_ids=list(range(N_CORES)))
    return assemble(LAST_RESULTS.results)


# revision 11
# speedup vs baseline: 45.0423x; 45.0423x over previous
"""DistMult edge scoring on 8 Trainium2 NeuronCores.

score[r, e] = sum_d x[src[r,e], d] * x[dst[r,e], d] * rel[r, d]

Strategy (edge-sharded, data-parallel):
  - Shard the 500k edges across 8 cores (62500 each); replicate the node
    table (51 MB) and rel embeddings in each core's DRAM.
  - Gathers use the batched GPSIMD dma_gather (one 512 B descriptor per
    row). Its indices are int16, so the host buckets each core's edges by
    (src_range, dst_range) pairs of 32768-row table ranges; within a
    bucket both gathers draw from a single range with local int16 indices.
  - Buckets are padded to the max size across cores (rounded to 128) so a
    single SPMD program serves all 8 cores; pad slots gather row 0 of the
    range and their scores are discarded on the host.
  - Per chunk of up to 4096 edges: gather xs, xd; DVE: v = xs*xd,
    u = v*rel (0-stride broadcast), grouped reduce_sum -> scores.
  - Chunk element t lands at (partition t%128, column t//128); the host
    inverts the whole permutation when assembling the output.
"""

import numpy as np

N_CORES = 8
N_NODES = 100000
DIM = 128
N_REL = 3
N_EDGES = 500000
E_CORE = N_EDGES // N_CORES          # 62500
RANGE = 32768                        # int16-addressable table range
N_RANGES = (N_NODES + RANGE - 1) // RANGE   # 4
N_BUCKETS = N_RANGES * N_RANGES      # 16
CHUNK_COLS = 32                      # max columns (128 edges each) per gather
GATHER_BUFS = 3                      # buffering depth for gather tiles

_CACHE = {}
LAST_RESULTS = None  # bass_utils.BassKernelResults from the most recent run


def _build_nc(caps, cols_max):
    """caps[r][k] = padded bucket capacity (multiple of 128) for relation r,
    bucket k = src_range*N_RANGES + dst_range. cols_max = max_r total cols."""
    from contextlib import ExitStack

    import concourse.bacc as bacc
    import concourse.mybir as mybir
    import concourse.tile as tile

    tot_s = sum(sum(c) for c in caps) // 16  # int16 image cols per side

    nc = bacc.Bacc("TRN2", target_bir_lowering=False, debug=False,
                   num_devices=N_CORES)
    table = nc.dram_tensor("table", (N_NODES, DIM), mybir.dt.float32,
                           kind="ExternalInput")
    relb = nc.dram_tensor("relb", (N_REL, 128, DIM), mybir.dt.float32,
                          kind="ExternalInput")
    sidx = nc.dram_tensor("sidx", (128, tot_s), mybir.dt.int16,
                          kind="ExternalInput")
    didx = nc.dram_tensor("didx", (128, tot_s), mybir.dt.int16,
                          kind="ExternalInput")
    out = nc.dram_tensor("out", (N_REL, 128, cols_max), mybir.dt.float32,
                         kind="ExternalOutput")

    with tile.TileContext(nc) as tc, ExitStack() as ctx:
        const_pool = ctx.enter_context(tc.tile_pool(name="const", bufs=2))
        ipool = ctx.enter_context(tc.tile_pool(name="idx", bufs=1))
        gpool = ctx.enter_context(tc.tile_pool(name="gather", bufs=GATHER_BUFS))
        spool = ctx.enter_context(tc.tile_pool(name="scores", bufs=2))

        # whole int16 index images stay SBUF-resident (~3 MB per side)
        sidx_sb = ipool.tile([128, tot_s], mybir.dt.int16, tag="sidx")
        didx_sb = ipool.tile([128, tot_s], mybir.dt.int16, tag="didx")
        nc.sync.dma_start(out=sidx_sb[:], in_=sidx[:, :])
        nc.sync.dma_start(out=didx_sb[:], in_=didx[:, :])

        img_off = 0  # running column offset into the int16 index images
        for r in range(N_REL):
            relb_sb = const_pool.tile([128, DIM], mybir.dt.float32, tag="relb")
            nc.sync.dma_start(out=relb_sb[:], in_=relb[r, :, :])
            cols_r = sum(caps[r]) // 128
            scores_sb = spool.tile([128, cols_max], mybir.dt.float32,
                                   tag="scores")
            col_off = 0
            for k in range(N_BUCKETS):
                cap = caps[r][k]
                if cap == 0:
                    continue
                si = (k // N_RANGES) * RANGE
                di = (k % N_RANGES) * RANGE
                s_tab = table[si : min(si + RANGE, N_NODES), :]
                d_tab = table[di : min(di + RANGE, N_NODES), :]
                cols_b = cap // 128
                for c0 in range(0, cols_b, CHUNK_COLS):
                    cc = min(CHUNK_COLS, cols_b - c0)
                    n = cc * 128
                    xs_t = gpool.tile([128, CHUNK_COLS * DIM],
                                      mybir.dt.float32, tag="xs")
                    xd_t = gpool.tile([128, CHUNK_COLS * DIM],
                                      mybir.dt.float32, tag="xd")
                    nc.gpsimd.dma_gather(
                        xs_t[:, : cc * DIM].rearrange("p (c d) -> p c d",
                                                      d=DIM),
                        s_tab, sidx_sb[:, img_off : img_off + cc * 8],
                        n, n, DIM, single_packet=False)
                    nc.gpsimd.dma_gather(
                        xd_t[:, : cc * DIM].rearrange("p (c d) -> p c d",
                                                      d=DIM),
                        d_tab, didx_sb[:, img_off : img_off + cc * 8],
                        n, n, DIM, single_packet=False)
                    img_off += cc * 8
                    nc.vector.tensor_tensor(
                        out=xd_t[:, : cc * DIM], in0=xs_t[:, : cc * DIM],
                        in1=xd_t[:, : cc * DIM], op=mybir.AluOpType.mult)
                    nc.vector.tensor_tensor(
                        out=xd_t[:, : cc * DIM], in0=xd_t[:, : cc * DIM],
                        in1=relb_sb[:, None, :].to_broadcast([128, cc, DIM]),
                        op=mybir.AluOpType.mult)
                    nc.vector.reduce_sum(
                        out=scores_sb[:, col_off : col_off + cc],
                        in_=xd_t[:, : cc * DIM].rearrange("p (c d) -> p c d",
                                                          d=DIM),
                        axis=mybir.AxisListType.X)
                    col_off += cc
            assert col_off == cols_r
            nc.sync.dma_start(out=out[r, :, :cols_r],
                              in_=scores_sb[:, :cols_r])

    nc.compile()
    return nc


def _pack_idx16(vals, cap):
    """Local indices [n] (n <= cap, cap % 128 == 0) -> replicated int16
    image [128, cap // 16], zero-padded."""
    a = np.zeros(cap, dtype=np.int16)
    a[: len(vals)] = vals
    return np.tile(a.reshape(-1, 16).T, (8, 1))  # [16, cap/16] -> [128, .]


SORT_MODE = "dst"  # none | src | dst | src_block_dst


def _bucket_order(s, d, b):
    """Permutation of edges grouped by bucket id b, with optional intra-bucket
    ordering for DRAM row-buffer locality on the gather streams."""
    if SORT_MODE == "none":
        return np.argsort(b, kind="stable")
    if SORT_MODE == "src":
        return np.lexsort((s, b))
    if SORT_MODE == "dst":
        return np.lexsort((d, b))
    if SORT_MODE == "src_block_dst":
        BS = 4096  # src block rows
        return np.lexsort((d, s // BS, b))
    raise ValueError(SORT_MODE)


def prepare(node_embeds, rel_emb, src_idx, dst_idx):
    """Bucket/pack host-side; returns (nc, in_maps, assemble) where
    assemble(results) -> full [N_REL, N_EDGES] scores."""
    node_embeds = np.ascontiguousarray(np.asarray(node_embeds,
                                                  dtype=np.float32))
    rel_emb = np.asarray(rel_emb, dtype=np.float32)
    src_idx = np.asarray(src_idx).astype(np.int64)
    dst_idx = np.asarray(dst_idx).astype(np.int64)

    relb = np.ascontiguousarray(
        np.broadcast_to(rel_emb[:, None, :], (N_REL, 128, DIM)))

    # ---- host-side bucketing ----
    # orders[c][r]: edge permutation (bucket-major); counts[c][r][k]
    orders = [[None] * N_REL for _ in range(N_CORES)]
    counts = np.zeros((N_CORES, N_REL, N_BUCKETS), dtype=np.int64)
    s_loc = [[None] * N_REL for _ in range(N_CORES)]
    d_loc = [[None] * N_REL for _ in range(N_CORES)]
    for c in range(N_CORES):
        lo = c * E_CORE
        for r in range(N_REL):
            s = src_idx[r, lo : lo + E_CORE]
            d = dst_idx[r, lo : lo + E_CORE]
            b = (s // RANGE) * N_RANGES + (d // RANGE)
            order = _bucket_order(s, d, b)
            orders[c][r] = order
            counts[c, r] = np.bincount(b, minlength=N_BUCKETS)
            s_loc[c][r] = (s[order] % RANGE).astype(np.int16)
            d_loc[c][r] = (d[order] % RANGE).astype(np.int16)

    caps = [[int(-(-counts[:, r, k].max() // 128) * 128)
             for k in range(N_BUCKETS)] for r in range(N_REL)]
    cols_max = max(sum(caps[r]) for r in range(N_REL)) // 128

    key = (tuple(map(tuple, caps)), cols_max)
    if _CACHE.get("key") != key:
        _CACHE["nc"] = _build_nc(caps, cols_max)
        _CACHE["key"] = key
    nc = _CACHE["nc"]

    # ---- pack index images (chunked exactly like the device loop) ----
    tot_s = sum(sum(c) for c in caps) // 16
    in_maps = []
    for c in range(N_CORES):
        s_img = np.empty((128, tot_s), dtype=np.int16)
        d_img = np.empty((128, tot_s), dtype=np.int16)
        img_off = 0
        for r in range(N_REL):
            u = 0  # position within this core's bucket-sorted edge stream
            for k in range(N_BUCKETS):
                cap = caps[r][k]
                if cap == 0:
                    continue
                cnt = int(counts[c, r, k])
                sv = s_loc[c][r][u : u + cnt]
                dv = d_loc[c][r][u : u + cnt]
                u += cnt
                # pad bucket to cap, then emit in CHUNK_COLS chunks
                sp = np.zeros(cap, dtype=np.int16); sp[:cnt] = sv
                dp = np.zeros(cap, dtype=np.int16); dp[:cnt] = dv
                for c0 in range(0, cap // 128, CHUNK_COLS):
                    cc = min(CHUNK_COLS, cap // 128 - c0)
                    n = cc * 128
                    seg = slice(c0 * 128, c0 * 128 + n)
                    s_img[:, img_off : img_off + cc * 8] = _pack_idx16(sp[seg], n)
                    d_img[:, img_off : img_off + cc * 8] = _pack_idx16(dp[seg], n)
                    img_off += cc * 8
        assert img_off == tot_s
        in_maps.append({"table": node_embeds, "relb": relb,
                        "sidx": s_img, "didx": d_img})

    def assemble(results):
        out = np.empty((N_REL, N_EDGES), dtype=np.float32)
        for c, res in enumerate(results):
            buf = res["out"]  # [N_REL, 128, cols_max]
            lo = c * E_CORE
            for r in range(N_REL):
                colmajor = buf[r].T.ravel()  # index = col*128 + partition
                u = 0
                off = 0
                for k in range(N_BUCKETS):
                    cap = caps[r][k]
                    if cap == 0:
                        continue
                    cnt = int(counts[c, r, k])
                    e_ids = orders[c][r][u : u + cnt]
                    out[r, lo + e_ids] = colmajor[off : off + cnt]
                    u += cnt
                    off += cap
        return out

    return nc, in_maps, assemble


def kernel(node_embeds, rel_emb, src_idx, dst_idx):
    global LAST_RESULTS
    from concourse import bass_utils

    nc, in_maps, assemble = prepare(node_embeds, rel_emb, src_idx, dst_idx)
    LAST_RESULTS = bass_utils.run_bass_kernel_spmd(
        nc, in_maps, core_ids=list(range(N_CORES)))
    return assemble(LAST_RESULTS.results)


# revision 12
# speedup vs baseline: 50.7317x; 1.1263x over previous
"""DistMult edge scoring on 8 Trainium2 NeuronCores.

score[r, e] = sum_d x[src[r,e], d] * x[dst[r,e], d] * rel[r, d]

Strategy (edge-sharded, data-parallel):
  - Shard the 500k edges across 8 cores (62500 each); replicate the node
    table (51 MB) and rel embeddings in each core's DRAM.
  - Gathers use the batched GPSIMD dma_gather (one 512 B descriptor per
    row). Its indices are int16, so the host buckets each core's edges by
    (src_range, dst_range) pairs of 32768-row table ranges; within a
    bucket both gathers draw from a single range with local int16 indices.
  - Buckets are padded to the max size across cores (rounded to 128) so a
    single SPMD program serves all 8 cores; pad slots gather row 0 of the
    range and their scores are discarded on the host.
  - Per chunk of up to 4096 edges: gather xs, xd; DVE: v = xs*xd,
    u = v*rel (0-stride broadcast), grouped reduce_sum -> scores.
  - Chunk element t lands at (partition t%128, column t//128); the host
    inverts the whole permutation when assembling the output.
"""

import numpy as np

N_CORES = 8
N_NODES = 100000
DIM = 128
N_REL = 3
N_EDGES = 500000
E_CORE = N_EDGES // N_CORES          # 62500
RANGE = 32768                        # int16-addressable table range
N_RANGES = (N_NODES + RANGE - 1) // RANGE   # 4
N_BUCKETS = N_RANGES * N_RANGES      # 16
CHUNK_COLS = 32                      # max columns (128 edges each) per gather
GATHER_BUFS = 3                      # buffering depth for gather tiles

_CACHE = {}
LAST_RESULTS = None  # bass_utils.BassKernelResults from the most recent run


def _build_nc(caps, cols_max):
    """caps[r][k] = padded bucket capacity (multiple of 128) for relation r,
    bucket k = src_range*N_RANGES + dst_range. cols_max = max_r total cols."""
    from contextlib import ExitStack

    import concourse.bacc as bacc
    import concourse.mybir as mybir
    import concourse.tile as tile

    tot_s = sum(sum(c) for c in caps) // 16  # int16 image cols per side

    nc = bacc.Bacc("TRN2", target_bir_lowering=False, debug=False,
                   num_devices=N_CORES)
    table = nc.dram_tensor("table", (N_NODES, DIM), mybir.dt.float32,
                           kind="ExternalInput")
    relb = nc.dram_tensor("relb", (N_REL, 128, DIM), mybir.dt.float32,
                          kind="ExternalInput")
    sidx = nc.dram_tensor("sidx", (128, tot_s), mybir.dt.int16,
                          kind="ExternalInput")
    didx = nc.dram_tensor("didx", (128, tot_s), mybir.dt.int16,
                          kind="ExternalInput")
    out = nc.dram_tensor("out", (N_REL, 128, cols_max), mybir.dt.float32,
                         kind="ExternalOutput")

    with tile.TileContext(nc) as tc, ExitStack() as ctx:
        const_pool = ctx.enter_context(tc.tile_pool(name="const", bufs=2))
        ipool = ctx.enter_context(tc.tile_pool(name="idx", bufs=1))
        gpool = ctx.enter_context(tc.tile_pool(name="gather", bufs=GATHER_BUFS))
        spool = ctx.enter_context(tc.tile_pool(name="scores", bufs=2))

        # whole int16 index images stay SBUF-resident (~3 MB per side)
        sidx_sb = ipool.tile([128, tot_s], mybir.dt.int16, tag="sidx")
        didx_sb = ipool.tile([128, tot_s], mybir.dt.int16, tag="didx")
        nc.sync.dma_start(out=sidx_sb[:], in_=sidx[:, :])
        nc.sync.dma_start(out=didx_sb[:], in_=didx[:, :])

        img_off = 0  # running column offset into the int16 index images
        for r in range(N_REL):
            relb_sb = const_pool.tile([128, DIM], mybir.dt.float32, tag="relb")
            nc.sync.dma_start(out=relb_sb[:], in_=relb[r, :, :])
            cols_r = sum(caps[r]) // 128
            scores_sb = spool.tile([128, cols_max], mybir.dt.float32,
                                   tag="scores")
            col_off = 0
            for k in range(N_BUCKETS):
                cap = caps[r][k]
                if cap == 0:
                    continue
                si = (k // N_RANGES) * RANGE
                di = (k % N_RANGES) * RANGE
                s_tab = table[si : min(si + RANGE, N_NODES), :]
                d_tab = table[di : min(di + RANGE, N_NODES), :]
                cols_b = cap // 128
                chunks = []
                for c0 in range(0, cols_b, CHUNK_COLS):
                    cc = min(CHUNK_COLS, cols_b - c0)
                    chunks.append((cc, img_off))
                    img_off += cc * 8
                # issue gathers grouped by stream (src runs, then dst runs):
                # back-to-back same-table gathers measurably beat alternating
                # src/dst. Group size capped below GATHER_BUFS so the xs tiles
                # of a group can all be live at once.
                for g0 in range(0, len(chunks), GATHER_BUFS - 1):
                    grp = chunks[g0 : g0 + GATHER_BUFS - 1]
                    xs_tiles = []
                    for cc, io in grp:
                        xs_t = gpool.tile([128, CHUNK_COLS * DIM],
                                          mybir.dt.float32, tag="xs")
                        nc.gpsimd.dma_gather(
                            xs_t[:, : cc * DIM].rearrange("p (c d) -> p c d",
                                                          d=DIM),
                            s_tab, sidx_sb[:, io : io + cc * 8],
                            cc * 128, cc * 128, DIM, single_packet=False)
                        xs_tiles.append(xs_t)
                    for j, (cc, io) in enumerate(grp):
                        xd_t = gpool.tile([128, CHUNK_COLS * DIM],
                                          mybir.dt.float32, tag="xd")
                        nc.gpsimd.dma_gather(
                            xd_t[:, : cc * DIM].rearrange("p (c d) -> p c d",
                                                          d=DIM),
                            d_tab, didx_sb[:, io : io + cc * 8],
                            cc * 128, cc * 128, DIM, single_packet=False)
                        xs_t = xs_tiles[j]
                        nc.vector.tensor_tensor(
                            out=xd_t[:, : cc * DIM], in0=xs_t[:, : cc * DIM],
                            in1=xd_t[:, : cc * DIM], op=mybir.AluOpType.mult)
                        nc.vector.tensor_tensor(
                            out=xd_t[:, : cc * DIM], in0=xd_t[:, : cc * DIM],
                            in1=relb_sb[:, None, :].to_broadcast(
                                [128, cc, DIM]),
                            op=mybir.AluOpType.mult)
                        nc.vector.reduce_sum(
                            out=scores_sb[:, col_off : col_off + cc],
                            in_=xd_t[:, : cc * DIM].rearrange(
                                "p (c d) -> p c d", d=DIM),
                            axis=mybir.AxisListType.X)
                        col_off += cc
            assert col_off == cols_r
            nc.sync.dma_start(out=out[r, :, :cols_r],
                              in_=scores_sb[:, :cols_r])

    nc.compile()
    return nc


def _pack_idx16(vals, cap):
    """Local indices [n] (n <= cap, cap % 128 == 0) -> replicated int16
    image [128, cap // 16], zero-padded."""
    a = np.zeros(cap, dtype=np.int16)
    a[: len(vals)] = vals
    return np.tile(a.reshape(-1, 16).T, (8, 1))  # [16, cap/16] -> [128, .]


SORT_MODE = "dst"  # none | src | dst | src_block_dst


def _bucket_order(s, d, b):
    """Permutation of edges grouped by bucket id b, with optional intra-bucket
    ordering for DRAM row-buffer locality on the gather streams."""
    if SORT_MODE == "none":
        return np.argsort(b, kind="stable")
    if SORT_MODE == "src":
        return np.lexsort((s, b))
    if SORT_MODE == "dst":
        return np.lexsort((d, b))
    if SORT_MODE == "src_block_dst":
        BS = 4096  # src block rows
        return np.lexsort((d, s // BS, b))
    raise ValueError(SORT_MODE)


def prepare(node_embeds, rel_emb, src_idx, dst_idx):
    """Bucket/pack host-side; returns (nc, in_maps, assemble) where
    assemble(results) -> full [N_REL, N_EDGES] scores."""
    node_embeds = np.ascontiguousarray(np.asarray(node_embeds,
                                                  dtype=np.float32))
    rel_emb = np.asarray(rel_emb, dtype=np.float32)
    src_idx = np.asarray(src_idx).astype(np.int64)
    dst_idx = np.asarray(dst_idx).astype(np.int64)

    relb = np.ascontiguousarray(
        np.broadcast_to(rel_emb[:, None, :], (N_REL, 128, DIM)))

    # ---- host-side bucketing ----
    # orders[c][r]: edge permutation (bucket-major); counts[c][r][k]
    orders = [[None] * N_REL for _ in range(N_CORES)]
    counts = np.zeros((N_CORES, N_REL, N_BUCKETS), dtype=np.int64)
    s_loc = [[None] * N_REL for _ in range(N_CORES)]
    d_loc = [[None] * N_REL for _ in range(N_CORES)]
    for c in range(N_CORES):
        lo = c * E_CORE
        for r in range(N_REL):
            s = src_idx[r, lo : lo + E_CORE]
            d = dst_idx[r, lo : lo + E_CORE]
            b = (s // RANGE) * N_RANGES + (d // RANGE)
            order = _bucket_order(s, d, b)
            orders[c][r] = order
            counts[c, r] = np.bincount(b, minlength=N_BUCKETS)
            s_loc[c][r] = (s[order] % RANGE).astype(np.int16)
            d_loc[c][r] = (d[order] % RANGE).astype(np.int16)

    caps = [[int(-(-counts[:, r, k].max() // 128) * 128)
             for k in range(N_BUCKETS)] for r in range(N_REL)]
    cols_max = max(sum(caps[r]) for r in range(N_REL)) // 128

    key = (tuple(map(tuple, caps)), cols_max)
    if _CACHE.get("key") != key:
        _CACHE["nc"] = _build_nc(caps, cols_max)
        _CACHE["key"] = key
    nc = _CACHE["nc"]

    # ---- pack index images (chunked exactly like the device loop) ----
    tot_s = sum(sum(c) for c in caps) // 16
    in_maps = []
    for c in range(N_CORES):
        s_img = np.empty((128, tot_s), dtype=np.int16)
        d_img = np.empty((128, tot_s), dtype=np.int16)
        img_off = 0
        for r in range(N_REL):
            u = 0  # position within this core's bucket-sorted edge stream
            for k in range(N_BUCKETS):
                cap = caps[r][k]
                if cap == 0:
                    continue
                cnt = int(counts[c, r, k])
                sv = s_loc[c][r][u : u + cnt]
                dv = d_loc[c][r][u : u + cnt]
                u += cnt
                # pad bucket to cap, then emit in CHUNK_COLS chunks
                sp = np.zeros(cap, dtype=np.int16); sp[:cnt] = sv
                dp = np.zeros(cap, dtype=np.int16); dp[:cnt] = dv
                for c0 in range(0, cap // 128, CHUNK_COLS):
                    cc = min(CHUNK_COLS, cap // 128 - c0)
                    n = cc * 128
                    seg = slice(c0 * 128, c0 * 128 + n)
                    s_img[:, img_off : img_off + cc * 8] = _pack_idx16(sp[seg], n)
                    d_img[:, img_off : img_off + cc * 8] = _pack_idx16(dp[seg], n)
                    img_off += cc * 8
        assert img_off == tot_s
        in_maps.append({"table": node_embeds, "relb": relb,
                        "sidx": s_img, "didx": d_img})

    def assemble(results):
        out = np.empty((N_REL, N_EDGES), dtype=np.float32)
        for c, res in enumerate(results):
            buf = res["out"]  # [N_REL, 128, cols_max]
            lo = c * E_CORE
            for r in range(N_REL):
                colmajor = buf[r].T.ravel()  # index = col*128 + partition
                u = 0
                off = 0
                for k in range(N_BUCKETS):
                    cap = caps[r][k]
                    if cap == 0:
                        continue
                    cnt = int(counts[c, r, k])
                    e_ids = orders[c][r][u : u + cnt]
                    out[r, lo + e_ids] = colmajor[off : off + cnt]
                    u += cnt
                    off += cap
        return out

    return nc, in_maps, assemble


def kernel(node_embeds, rel_emb, src_idx, dst_idx):
    global LAST_RESULTS
    from concourse import bass_utils

    nc, in_maps, assemble = prepare(node_embeds, rel_emb, src_idx, dst_idx)
    LAST_RESULTS = bass_utils.run_bass_kernel_spmd(
        nc, in_maps, core_ids=list(range(N_CORES)))
    return assemble(LAST_RESULTS.results)
